# revision 19
# baseline (speedup 1.0000x reference)
"""CrossModalAttention Trainium2 kernel.

Per-core (data-parallel over batch B=8 -> 8 NeuronCores):
  y_b = softmax((x_b Wq)(x_b Wk)^T * SCALE * (1 + mask_b)) (x_b Wv) @ Wo + bo

Design (transposed-softmax layout, no big transposes of the score matrix):
  - Host folds SCALE into Wq and precomputes M1 = 1+mask in bf16 (single
    half by default; optional hi/lo split via KB_M1ONE=0) so the mask can
    be moved through the DMA-transpose xbar (2-byte dtype only).
  - Scores are computed transposed, sT[j, i] = K^T-row-packed bf16 matmuls
    (two heads per PE pass via row tiling at partitions 0/64).
  - Softmax runs without max-subtraction (|t| <= ~13, exp is safe):
    DVE multiplies sT(PSUM f32) by M1T writing fp16, ACT exponentiates
    fp16->bf16 at 2x rate, and the AV matmul uses a ones-augmented V
    (65th column) so the softmax denominators fall out of the same
    accumulation for free (row 64).
  - Normalization: DVE fast-approx reciprocal of the rowsum row, GPSIMD
    partition_broadcast replicates it across 64 partitions in SBUF,
    DVE multiply writes outT (bf16).
  - y = outT^T-contracted bf16 matmul against Wo with bias folded in via
    a K=1 ones x bo outer product into the same PSUM accumulation.
  - Mask DMA-transposes for i-chunk ic+1 are prefetched during chunk ic's
    head loop; chunk 0's are issued before the projection phase.
"""

import numpy as np

B, N, D = 8, 2048, 512
H, DH = 8, 64
SCALE = DH ** -0.5

IC_N, IC = 4, 512      # i-chunks
JT_N, JT = 16, 128     # j-tiles
P_N = 4                # head pairs

_built = {}


def _flags():
    import os
    return {
        "M1ONE": os.environ.get("KB_M1ONE", "1") == "1",
        "QKBF": os.environ.get("KB_QKBF", "1") == "1",
        "F16": os.environ.get("KB_F16", "1") == "1",
        "RCPFAST": os.environ.get("KB_RCPFAST", "1") == "1",
        "RCPSRC": os.environ.get("KB_RCPSRC", "sbuf"),
        "PBCAST": os.environ.get("KB_PBCAST", "1") == "1",
        "YPSUM": os.environ.get("KB_YPSUM", "0") == "1",
        "HOIST": os.environ.get("KB_HOIST", "1") == "1",
        "EXPG": os.environ.get("KB_EXPG", "1") == "1",
        "PTT": int(os.environ.get("KB_PTT", "0")),
        "ZDVE": os.environ.get("KB_ZDVE", "0") == "1",
    }


def _build():
    fl = _flags()
    M1ONE, QKBF, F16 = fl["M1ONE"], fl["QKBF"], fl["F16"]
    RCPFAST, PBCAST, YPSUM, HOIST = (
        fl["RCPFAST"], fl["PBCAST"], fl["YPSUM"], fl["HOIST"])
    RCPSRC = fl["RCPSRC"]
    EXPG, PTT, ZDVE = fl["EXPG"], fl["PTT"], fl["ZDVE"]

    import concourse.tile as tile
    from concourse import bacc, mybir
    from concourse.masks import make_identity

    F32 = mybir.dt.float32
    F32R = mybir.dt.float32r
    BF16 = mybir.dt.bfloat16
    FP16 = mybir.dt.float16
    QK_DT = BF16 if QKBF else F32R
    E_DT = BF16 if F16 else F32R
    T_DT = FP16 if F16 else F32
    WO_DT = BF16 if F16 else F32R
    O_DT = BF16 if F16 else F32R
    Exp = mybir.ActivationFunctionType.Exp
    MULT = mybir.AluOpType.mult
    ADD = mybir.AluOpType.add

    nc = bacc.Bacc()
    x_d = nc.declare_dram_parameter("x", [N, D], F32, isOutput=False)
    m1h_d = nc.declare_dram_parameter("m1h", [N, N], BF16, isOutput=False)
    if not M1ONE:
        m1l_d = nc.declare_dram_parameter("m1l", [N, N], BF16, isOutput=False)
    wq_d = nc.declare_dram_parameter("wq", [D, D], F32R, isOutput=False)
    wk_d = nc.declare_dram_parameter("wk", [D, D], F32R, isOutput=False)
    wv_d = nc.declare_dram_parameter("wv", [D, D], F32R, isOutput=False)
    wo_d = nc.declare_dram_parameter("wo", [D, D], WO_DT, isOutput=False)
    bo_d = nc.declare_dram_parameter("bo", [1, D], F32R, isOutput=False)
    y_d = nc.declare_dram_parameter("y", [N, D], F32, isOutput=True)

    with nc.allow_low_precision(reason="bf16/f32r matmul pipeline"), \
         tile.TileContext(nc) as tc:
        with tc.tile_pool(name="persist", bufs=1) as pp:
            ident = pp.tile([128, 128], F32, tag="ident")
            make_identity(nc, ident)
            ones_f = pp.tile([128, 128], F32, tag="ones_f")
            nc.vector.memset(ones_f, 1.0)
            ones_r = pp.tile([128, 128], F32R, tag="ones_r")
            nc.vector.tensor_copy(ones_r[:], ones_f[:])

            qT = [pp.tile([128, N], QK_DT, tag=f"qT{t}", name=f"qT{t}") for t in range(4)]
            kT = [pp.tile([128, N], QK_DT, tag=f"kT{t}", name=f"kT{t}") for t in range(4)]
            v_sb = [pp.tile([128, H * 65], E_DT, tag=f"v{t}", name=f"v{t}")
                    for t in range(JT_N)]
            outT = [pp.tile([128, N], O_DT, tag=f"oT{t}", name=f"oT{t}")
                    for t in range(4)]

            # mask tiles: transposed [j within jt, i] per (jt); double-buffered
            # across ic so ic+1's DMA-transposes overlap ic's head loop.
            def issue_mask_dmas(ic):
                ms = []
                for jt in range(JT_N):
                    mh = pp.tile([128, IC], BF16, tag=f"mh{jt}", bufs=2,
                                 name=f"mh{jt}")
                    nc.sync.dma_start(
                        out=mh,
                        in_=m1h_d[ic * IC:(ic + 1) * IC, jt * 128:(jt + 1) * 128],
                        transpose=True)
                    if not M1ONE:
                        ml = pp.tile([128, IC], BF16, tag=f"ml{jt}", bufs=2,
                                     name=f"ml{jt}")
                        nc.sync.dma_start(
                            out=ml,
                            in_=m1l_d[ic * IC:(ic + 1) * IC, jt * 128:(jt + 1) * 128],
                            transpose=True)
                        m1 = pp.tile([128, IC], F32, tag=f"m1{jt}", bufs=2,
                                     name=f"m1{jt}")
                        nc.gpsimd.tensor_tensor(out=m1[:], in0=mh[:], in1=ml[:], op=ADD)
                        ms.append(m1)
                    else:
                        ms.append(mh)
                return ms

            masks = {}
            if HOIST:
                masks[0] = issue_mask_dmas(0)

            # ---------------- phase 0 + 1: xT, projections ----------------
            with tc.tile_pool(name="ph01", bufs=1) as p1, \
                 tc.tile_pool(name="ph01ps", bufs=1, space="PSUM") as p1p:
                wq_sb = [p1.tile([128, D], F32R, tag=f"wq{c}", name=f"wq{c}") for c in range(4)]
                wk_sb = [p1.tile([128, D], F32R, tag=f"wk{c}", name=f"wk{c}") for c in range(4)]
                wv_sb = [p1.tile([128, D], F32R, tag=f"wv{c}", name=f"wv{c}") for c in range(4)]
                for c in range(4):
                    nc.gpsimd.dma_start(out=wq_sb[c], in_=wq_d[c * 128:(c + 1) * 128, :])
                    nc.gpsimd.dma_start(out=wk_sb[c], in_=wk_d[c * 128:(c + 1) * 128, :])
                    nc.gpsimd.dma_start(out=wv_sb[c], in_=wv_d[c * 128:(c + 1) * 128, :])

                xT = [p1.tile([128, N], F32R, tag=f"xT{c}", name=f"xT{c}") for c in range(4)]
                for nt in range(JT_N):
                    x_sb = p1.tile([128, D], F32, tag="x_sb", bufs=3, name="x_sb")
                    nc.gpsimd.dma_start(out=x_sb, in_=x_d[nt * 128:(nt + 1) * 128, :])
                    tpp = p1p.tile([128, 512], F32, tag="tpp", bufs=2, name="tpp")
                    for c in range(4):
                        nc.tensor.transpose(tpp[:, c * 128:(c + 1) * 128],
                                            x_sb[:, c * 128:(c + 1) * 128], ident[:])
                    for c in range(4):
                        nc.scalar.copy(xT[c][:, nt * 128:(nt + 1) * 128],
                                       tpp[:, c * 128:(c + 1) * 128])

                for w_sb, dstT in ((wq_sb, qT), (wk_sb, kT)):
                    for hdt in range(4):
                        for nch in range(4):
                            qp = p1p.tile([128, 512], F32, tag="qp", bufs=2, name="qp")
                            for c in range(4):
                                nc.tensor.matmul(
                                    qp[:], w_sb[c][:, hdt * 128:(hdt + 1) * 128],
                                    xT[c][:, nch * 512:(nch + 1) * 512],
                                    start=(c == 0), stop=(c == 3))
                            nc.scalar.copy(dstT[hdt][:, nch * 512:(nch + 1) * 512], qp[:])

                for nt in range(JT_N):
                    vp = p1p.tile([128, 512], F32, tag="vp", bufs=2, name="vp")
                    for c in range(4):
                        nc.tensor.matmul(vp[:], xT[c][:, nt * 128:(nt + 1) * 128],
                                         wv_sb[c][:], start=(c == 0), stop=(c == 3))
                    vdst = v_sb[nt].rearrange("p (h e) -> p h e", e=65)
                    nc.vector.tensor_copy(vdst[:, :, 0:64],
                                          vp[:].rearrange("p (h e) -> p h e", e=64))
                    nc.vector.tensor_copy(vdst[:, :, 64:65],
                                          ones_r[:, 0:H].rearrange("p (h e) -> p h e", e=1))

            # ---------------- phase 2: attention (+ per-ic y emission) ----------------
            wo_sb = [pp.tile([128, D], WO_DT, tag=f"wo{c}", name=f"wo{c}") for c in range(4)]
            for c in range(4):
                nc.gpsimd.dma_start(out=wo_sb[c], in_=wo_d[c * 128:(c + 1) * 128, :])
            bo_sb = pp.tile([1, D], F32R, tag="bo", name="bo")
            nc.gpsimd.dma_start(out=bo_sb, in_=bo_d[:])
            with tc.tile_pool(name="ph2", bufs=1) as p2, \
                 tc.tile_pool(name="ph2ps", bufs=1, space="PSUM") as p2p:
                for ic in range(IC_N):
                    if ic in masks:
                        m1T = masks.pop(ic)
                    else:
                        m1T = issue_mask_dmas(ic)
                    if ic + 1 < IC_N:
                        masks[ic + 1] = issue_mask_dmas(ic + 1)

                    for p in range(P_N):
                        av0 = p2p.tile([65, 512], F32, tag="av0", name="av0")
                        av1 = p2p.tile([65, 512], F32, tag="av1", name="av1")
                        # software pipeline over j-tile PAIRS: the AV matmuls
                        # trail the score/mask/exp chain by one pair so the PE
                        # never stalls on exp. The two mask-mults of a pair
                        # write one [128,2048] t-tile so a single ACT exp
                        # covers both (halves ACT instruction overhead).
                        LAGP = 1
                        NPAIR = JT_N // 2
                        egs = {}
                        for jp in range(NPAIR + LAGP):
                            if jp < NPAIR:
                                if EXPG:
                                    t_g = p2.tile([128, 2048], T_DT, tag="t_g",
                                                  bufs=2, name="t_g")
                                for k in range(2):
                                    jt = 2 * jp + k
                                    sp = p2p.tile([128, 1024], F32, tag="sp",
                                                  bufs=2, name="sp")
                                    nc.tensor.matmul(
                                        sp[:, 0:512],
                                        kT[p][0:64, jt * 128:(jt + 1) * 128],
                                        qT[p][0:64, ic * IC:(ic + 1) * IC],
                                        start=True, stop=True, tile_position=(0, 0))
                                    nc.tensor.matmul(
                                        sp[:, 512:1024],
                                        kT[p][64:128, jt * 128:(jt + 1) * 128],
                                        qT[p][64:128, ic * IC:(ic + 1) * IC],
                                        start=True, stop=True, tile_position=(64, 0))
                                    if not EXPG:
                                        t_g = p2.tile([128, 1024], T_DT, tag="t_g",
                                                      bufs=3, name="t_g")
                                        tdst = t_g[:]
                                    else:
                                        tdst = t_g[:, k * 1024:(k + 1) * 1024]
                                    eng = (nc.gpsimd if (PTT and jt % PTT == PTT - 1)
                                           else nc.vector)
                                    eng.tensor_tensor(
                                        out=tdst.rearrange("p (h i) -> p h i", h=2),
                                        in0=sp[:].rearrange("p (h i) -> p h i", h=2),
                                        in1=m1T[jt][:, None, :].broadcast_to((128, 2, IC)),
                                        op=MULT)
                                    if not EXPG:
                                        e_g = p2.tile([128, 1024], E_DT, tag="e_g",
                                                      bufs=4, name="e_g")
                                        nc.scalar.activation(e_g[:], t_g[:], Exp)
                                        egs[jt] = e_g
                                if EXPG:
                                    e_g = p2.tile([128, 2048], E_DT, tag="e_g",
                                                  bufs=LAGP + 2, name="e_g")
                                    nc.scalar.activation(e_g[:], t_g[:], Exp)
                                    egs[jp] = e_g
                            if jp >= LAGP:
                                j0p = jp - LAGP
                                for k in range(2):
                                    j0 = 2 * j0p + k
                                    if EXPG:
                                        e0 = egs[j0p]
                                        eslice = lambda h: e0[:, (k * 2 + h) * 512:
                                                              (k * 2 + h + 1) * 512]
                                    else:
                                        e0 = egs[j0]
                                        eslice = lambda h: e0[:, h * 512:(h + 1) * 512]
                                    for h in range(2):
                                        hh = 2 * p + h
                                        nc.tensor.matmul(
                                            (av0 if h == 0 else av1)[:],
                                            v_sb[j0][:, hh * 65:(hh + 1) * 65],
                                            eslice(h),
                                            start=(j0 == 0), stop=(j0 == JT_N - 1))
                                if EXPG:
                                    egs.pop(j0p)
                                else:
                                    egs.pop(2 * j0p)
                                    egs.pop(2 * j0p + 1)
                        if PBCAST:
                            # base-0 APs throughout; av PSUM feeds the TT
                            # directly (no tmp copies, no PE broadcast).
                            rcp0 = p2.tile([1, 512], F32, tag="rcp0", bufs=2, name="rcp0")
                            rcp1 = p2.tile([1, 512], F32, tag="rcp1", bufs=2, name="rcp1")
                            if RCPSRC == "sbuf":
                                z0_t = p2.tile([1, 512], F32, tag="z0", bufs=2, name="z0")
                                z1_t = p2.tile([1, 512], F32, tag="z1", bufs=2, name="z1")
                                if ZDVE:
                                    nc.vector.tensor_copy(z0_t[:], av0[64:65, :])
                                    nc.vector.tensor_copy(z1_t[:], av1[64:65, :])
                                else:
                                    nc.scalar.copy(z0_t[:], av0[64:65, :])
                                    nc.scalar.copy(z1_t[:], av1[64:65, :])
                                z0, z1 = z0_t[:], z1_t[:]
                            else:
                                z0, z1 = av0[64:65, :], av1[64:65, :]
                            if RCPFAST:
                                nc.vector.reciprocal_approx_fast(rcp0[:], z0)
                                nc.vector.reciprocal_approx_fast(rcp1[:], z1)
                            else:
                                nc.vector.reciprocal(rcp0[:], z0)
                                nc.vector.reciprocal(rcp1[:], z1)
                            bc0_t = p2.tile([64, 512], F32, tag="bc0s", bufs=2, name="bc0s")
                            bc1_t = p2.tile([64, 512], F32, tag="bc1s", bufs=2, name="bc1s")
                            nc.gpsimd.partition_broadcast(bc0_t[:], rcp0[:], channels=64)
                            nc.gpsimd.partition_broadcast(bc1_t[:], rcp1[:], channels=64)
                            nc.vector.tensor_tensor(
                                out=outT[p][0:64, ic * IC:(ic + 1) * IC],
                                in0=av0[0:64, :], in1=bc0_t[:], op=MULT)
                            nc.vector.tensor_tensor(
                                out=outT[p][64:128, ic * IC:(ic + 1) * IC],
                                in0=av1[0:64, :], in1=bc1_t[:], op=MULT)
                        else:
                            rcp = p2.tile([128, 512], F32R, tag="rcp", bufs=2, name="rcp")
                            nc.vector.reciprocal(rcp[64:65, :], av0[64:65, :])
                            nc.vector.reciprocal(rcp[32:33, :], av1[64:65, :])
                            tmp = p2.tile([128, 512], F32, tag="tmp", bufs=2, name="tmp")
                            nc.scalar.copy(tmp[0:64, :], av0[0:64, :])
                            nc.scalar.copy(tmp[64:128, :], av1[0:64, :])
                            bc0_t = p2p.tile([64, 512], F32, tag="bc0", name="bc0")
                            bc1_t = p2p.tile([64, 512], F32, tag="bc1", name="bc1")
                            nc.tensor.matmul(bc0_t[:], ones_r[64:65, 0:64], rcp[64:65, :],
                                             start=True, stop=True)
                            nc.tensor.matmul(bc1_t[:], ones_r[32:33, 0:64], rcp[32:33, :],
                                             start=True, stop=True)
                            nc.vector.tensor_tensor(
                                out=outT[p][0:64, ic * IC:(ic + 1) * IC],
                                in0=tmp[0:64, :], in1=bc0_t[:], op=MULT)
                            nc.vector.tensor_tensor(
                                out=outT[p][64:128, ic * IC:(ic + 1) * IC],
                                in0=tmp[64:128, :], in1=bc1_t[:], op=MULT)

                    for itl in range(4):
                        it = ic * 4 + itl
                        if PBCAST:
                            yp_tag = "yp0" if itl % 2 == 0 else "yp1"
                        else:
                            yp_tag = "bc0" if itl % 2 == 0 else "bc1"
                        yp = p2p.tile([128, 512], F32, tag=yp_tag, name="yp")
                        nc.tensor.matmul(yp[:], ones_r[0:1, 0:128], bo_sb[0:1, :],
                                         start=True, stop=False)
                        for hdt in range(4):
                            nc.tensor.matmul(yp[:], outT[hdt][:, it * 128:(it + 1) * 128],
                                             wo_sb[hdt][:], start=False, stop=(hdt == 3))
                        if YPSUM:
                            nc.gpsimd.dma_start(out=y_d[it * 128:(it + 1) * 128, :], in_=yp[:])
                        else:
                            y_sb = p2.tile([128, D], F32, tag="y_sb", bufs=2, name="y_sb")
                            nc.scalar.copy(y_sb[:], yp[:])
                            nc.gpsimd.dma_start(out=y_d[it * 128:(it + 1) * 128, :], in_=y_sb[:])

    nc.finalize()
    return nc


def _get_nc():
    if "nc" not in _built:
        _built["nc"] = _build()
    return _built["nc"]


def _prep_inputs(x, mask, Wq, Wk, Wv, Wo, bo):
    import ml_dtypes
    fl = _flags()
    x = np.asarray(x, dtype=np.float32)
    mask = np.asarray(mask, dtype=np.float32)
    m1 = 1.0 + mask
    m1h = m1.astype(ml_dtypes.bfloat16)
    wq = (np.asarray(Wq, dtype=np.float32) * SCALE)
    wk = np.asarray(Wk, dtype=np.float32)
    wv = np.asarray(Wv, dtype=np.float32)
    wo = np.asarray(Wo, dtype=np.float32)
    if fl["F16"]:
        wo = wo.astype(ml_dtypes.bfloat16)
    bo2 = np.asarray(bo, dtype=np.float32).reshape(1, D)
    in_maps = []
    for b in range(x.shape[0]):
        im = {"x": x[b], "m1h": m1h[b],
              "wq": wq, "wk": wk, "wv": wv, "wo": wo, "bo": bo2}
        if not fl["M1ONE"]:
            im["m1l"] = (m1[b] - m1h[b].astype(np.float32)).astype(ml_dtypes.bfloat16)
        in_maps.append(im)
    return in_maps


def kernel(x, mask, Wq, Wk, Wv, Wo, bo):
    from concourse.bass_utils import run_bass_kernel_spmd

    nc = _get_nc()
    in_maps = _prep_inputs(x, mask, Wq, Wk, Wv, Wo, bo)
    res = run_bass_kernel_spmd(nc, in_maps, list(range(B)))
    return np.stack([res.results[b]["y"] for b in range(B)], axis=0)
